# revision 3
# baseline (speedup 1.0000x reference)
"""Trainium2 Bass kernel for nn_LowPassFilter (time-varying 9-tap windowed-sinc).

Math (matches reference.py):
  t in [0, N+HS):  ang = fl32(beta * t)           (f32 product rounding replicated!)
  s = sin(ang);  c = C0 + C1*s   (C0 = 4*pi^2, C1 = alpha*4000*pi)
  taps: filt[4] = 2c, filt[4+-m] = kappa_m * sin(2*pi*m*c),  kappa_m = w_{4+m}/(pi*m)
  out[t] = (2c*x[t] + sum_m filt_m*(x[t-m]+x[t+m])) / (2c + 2*sum_m filt_m)
Multiple angles from ONE pair of LUT sins (Sin LUT valid only on [-pi, pi]):
  f = c - round(c);  S1 = sin(2*pi*f) = sin(2*pi*c);  T = sin(pi*f)
  cos(2*pi*c) = 1 - 2*T^2;  sin(4*pi*c) = 2*S1*cos;  sin(6*pi*c) = S1*(3-4*S1^2)

Sharding: 1-D sequence parallel, 8 cores x 500_000 outputs (core 7: +4 tail),
halos passed from host (full input available). Layout [128 partitions x F=3968],
t_local = p*F + j, processed in 4 free-dim chunks of 992.
"""

import math
import numpy as np

# ---------------- problem constants (hardcoded per contract) ----------------
N = 4_000_000
HS = 4
NOUT = N + HS
NCORES = 8
KPC = N // NCORES            # 500_000 outputs per core (core 7 gets +HS tail)
P = 128
F = 3968                     # per-partition free size: 128*F = 507_904 >= 500_004
CH = 992                     # chunk of free dim
NCH = F // CH                # 4
CUTOFF = 1000.0
FS = 8000.0

MAGIC = 12582912.0           # 1.5 * 2**23, round-to-int magic for |v| < 2**22
C0 = float(np.float32(4.0 * math.pi * math.pi))
INV2PI = float(np.float32(1.0 / (2.0 * math.pi)))
PI_F = float(np.float32(math.pi))
TWO_PI_F = float(np.float32(2.0 * math.pi))

_W5 = math.sin(5.0 * math.pi / 8.0) ** 2     # 0.853553...
_W6 = 0.5
_W7 = math.sin(7.0 * math.pi / 8.0) ** 2     # 0.146446...
K1 = _W5 / math.pi
K2 = _W6 / (2.0 * math.pi)
K3 = _W7 / (3.0 * math.pi)
KG = float(np.float32(K1 + 2.0 * K2 + 3.0 * K3))
SQ2 = float(np.float32(2.0 * math.sqrt(K2)))  # Square(SQ2*T)   = 4*K2*T^2
SQ3 = float(np.float32(2.0 * math.sqrt(K3)))  # Square(SQ3*S1)  = 4*K3*S1^2
K1_F = float(np.float32(K1))
K2x2 = float(np.float32(2.0 * K2))
K3x3 = float(np.float32(3.0 * K3))

# Cody-Waite 3-term split of 2*pi (11-bit chunks: k <= 6366 < 2^13 keeps k*cw exact)
def _split_f32(v, bits):
    f = np.float32(v)
    m, e = math.frexp(float(f))
    scale = 2.0 ** (e - bits)
    hi = math.floor(float(f) / scale) * scale
    return float(np.float32(hi))

_TWO_PI = 2.0 * math.pi
CW1 = _split_f32(_TWO_PI, 11)
CW2 = _split_f32(_TWO_PI - CW1, 11)
CW3 = float(np.float32(_TWO_PI - CW1 - CW2))

_PROGRAM = None   # (nc,) built once per process
LAST_EXEC_NS = None
LAST_RESULTS = None


def _build_program():
    import concourse.bacc as bacc
    import concourse.mybir as mybir
    from concourse.tile import TileContext

    dt = mybir.dt.float32
    Alu = mybir.AluOpType
    Act = mybir.ActivationFunctionType

    nc = bacc.Bacc(None, target_bir_lowering=False, debug=False)

    xw = nc.dram_tensor("xw", [P, F + 6], dt, kind="ExternalInput")
    t0c = nc.dram_tensor("t0c", [P, 1], dt, kind="ExternalInput")
    c1c = nc.dram_tensor("c1c", [P, 1], dt, kind="ExternalInput")
    bc = nc.dram_tensor("bc", [P, 1], dt, kind="ExternalInput")
    yo = nc.dram_tensor("yo", [P, F], dt, kind="ExternalOutput")

    with TileContext(nc) as tc:
        with (
            tc.tile_pool(name="const", bufs=1) as cpool,
            tc.tile_pool(name="work", bufs=2) as pool,
        ):
            xt = cpool.tile([P, F + 6], dt, tag="xt", name="xt")
            nc.sync.dma_start(xt[:], xw[:])
            t0t = cpool.tile([P, 1], dt, tag="t0t", name="t0t")
            nc.sync.dma_start(t0t[:], t0c[:])
            c1t = cpool.tile([P, 1], dt, tag="c1t", name="c1t")
            nc.sync.dma_start(c1t[:], c1c[:])
            bt = cpool.tile([P, 1], dt, tag="bt", name="bt")
            nc.sync.dma_start(bt[:], bc[:])

            for ic in range(NCH):
                j0 = ic * CH

                TAIL = {"negG", "DhS", "Dh", "r0", "e1", "e2", "e3",
                        "nP2", "nP3", "W1", "Wt", "Z", "Y", "NUM", "o"}

                def tile(tag):
                    return pool.tile([P, CH], dt, tag=tag, name=tag,
                                     bufs=1 if tag in TAIL else 2)

                # t_local = p*F + (j0 + jj), exact int32 -> f32
                ti = pool.tile([P, CH], mybir.dt.int32, tag="ti", name="ti", bufs=1)
                nc.gpsimd.iota(ti[:], pattern=[[1, CH]], base=j0, channel_multiplier=F)
                tf = tile("tf")
                nc.vector.tensor_copy(tf[:], ti[:])

                # ang = fl32(beta * t)  (t = t_local + t0 exact in f32)
                ang = tile("ang")
                nc.vector.tensor_scalar(ang[:], tf[:], t0t[:, 0:1], bt[:, 0:1],
                                        Alu.add, Alu.mult)
                # k = round(ang / 2pi) via magic
                k1t = tile("k1")
                nc.vector.tensor_scalar(k1t[:], ang[:], INV2PI, MAGIC,
                                        Alu.mult, Alu.add)
                kf = tile("kf")
                nc.scalar.activation(kf[:], k1t[:], Act.Copy, bias=-MAGIC)
                # r = ((ang - k*CW1) - k*CW2) - k*CW3  in [-pi, pi]
                r = tile("r")
                nc.vector.cody_waite_cascade(r[:], ang[:], kf[:], CW1, CW2, CW3)
                s = tile("s")
                nc.scalar.activation(s[:], r[:], Act.Sin)
                # c = C0 + C1*s ; f = c - round(c)
                c = tile("c")
                nc.vector.tensor_scalar(c[:], s[:], c1t[:, 0:1], C0,
                                        Alu.mult, Alu.add)
                rk = tile("rk")
                nc.vector.tensor_scalar(rk[:], c[:], MAGIC, MAGIC,
                                        Alu.add, Alu.subtract)
                f = tile("f")
                nc.vector.tensor_tensor(f[:], c[:], rk[:], Alu.subtract)
                # trig of f
                T = tile("T")
                nc.scalar.activation(T[:], f[:], Act.Sin, scale=PI_F)
                S1 = tile("S1")
                nc.scalar.activation(S1[:], f[:], Act.Sin, scale=TWO_PI_F)
                Up = tile("Up")      # 4*K2*sin(pi f)^2
                nc.scalar.activation(Up[:], T[:], Act.Square, scale=SQ2)
                Qp = tile("Qp")      # 4*K3*sin(2pi f)^2
                nc.scalar.activation(Qp[:], S1[:], Act.Square, scale=SQ3)

                # negG = (Up - KG) + Qp = -(K1 + A2 + A3)
                negG = tile("negG")
                nc.vector.scalar_tensor_tensor(negG[:], Up[:], KG, Qp[:],
                                               Alu.subtract, Alu.add)
                DhS = tile("DhS")
                nc.vector.tensor_tensor(DhS[:], S1[:], negG[:], Alu.mult)
                Dh = tile("Dh")
                nc.vector.tensor_tensor(Dh[:], c[:], DhS[:], Alu.subtract)
                r0 = tile("r0")
                nc.vector.reciprocal_approx_fast(r0[:], Dh[:])

                # x side
                e1 = tile("e1")
                nc.vector.tensor_tensor(e1[:], xt[:, j0 + 4:j0 + 4 + CH],
                                        xt[:, j0 + 2:j0 + 2 + CH], Alu.add)
                e2 = tile("e2")
                nc.vector.tensor_tensor(e2[:], xt[:, j0 + 5:j0 + 5 + CH],
                                        xt[:, j0 + 1:j0 + 1 + CH], Alu.add)
                e3 = tile("e3")
                nc.vector.tensor_tensor(e3[:], xt[:, j0 + 6:j0 + 6 + CH],
                                        xt[:, j0 + 0:j0 + 0 + CH], Alu.add)
                nP2 = tile("nP2")    # (Up - 2K2)*e2 = -A2*e2
                nc.vector.scalar_tensor_tensor(nP2[:], Up[:], K2x2, e2[:],
                                               Alu.subtract, Alu.mult)
                nP3 = tile("nP3")    # (Qp - 3K3)*e3 = -A3*e3
                nc.vector.scalar_tensor_tensor(nP3[:], Qp[:], K3x3, e3[:],
                                               Alu.subtract, Alu.mult)
                W1 = tile("W1")      # K1*e1 + A2*e2
                nc.vector.scalar_tensor_tensor(W1[:], e1[:], K1_F, nP2[:],
                                               Alu.mult, Alu.subtract)
                Wt = tile("Wt")      # + A3*e3
                nc.vector.tensor_tensor(Wt[:], W1[:], nP3[:], Alu.subtract)
                Z = tile("Z")
                nc.vector.tensor_tensor(Z[:], S1[:], Wt[:], Alu.mult)
                Y = tile("Y")
                nc.vector.tensor_tensor(Y[:], c[:], xt[:, j0 + 3:j0 + 3 + CH],
                                        Alu.mult)
                NUM = tile("NUM")    # 0.5*Z + Y
                nc.vector.scalar_tensor_tensor(NUM[:], Z[:], 0.5, Y[:],
                                               Alu.mult, Alu.add)
                o = tile("o")
                nc.vector.tensor_tensor(o[:], NUM[:], r0[:], Alu.mult)
                nc.sync.dma_start(yo[:, j0:j0 + CH], o[:])

    nc.compile()
    return nc


def _get_program():
    global _PROGRAM
    if _PROGRAM is None:
        _PROGRAM = _build_program()
    return _PROGRAM


def kernel(x, alpha, beta, _trace=False, _trace_cores=None):
    global LAST_EXEC_NS, LAST_RESULTS
    from concourse.bass_utils import run_bass_kernel_spmd

    x = np.asarray(x, dtype=np.float32).reshape(-1)
    assert x.shape[0] == N, x.shape
    a64 = float(np.float32(np.asarray(alpha).reshape(())))
    b64 = float(np.float32(np.asarray(beta).reshape(())))
    C1 = float(np.float32(a64 * 4000.0 * math.pi))

    nc = _get_program()

    # host-side shard prep: windows with halos (halo exchange via full input)
    xp = np.zeros(3 + N + (NCORES * 0 + P * F + 6), dtype=np.float32)
    xp[3:3 + N] = x
    sw = np.lib.stride_tricks.sliding_window_view(xp, F + 6)
    c1col = np.full((P, 1), C1, dtype=np.float32)
    bcol = np.full((P, 1), np.float32(b64), dtype=np.float32)
    in_maps = []
    for core in range(NCORES):
        t0 = core * KPC
        rows = np.ascontiguousarray(sw[t0 + np.arange(P) * F])   # [P, F+6]
        in_maps.append({
            "xw": rows,
            "t0c": np.full((P, 1), np.float32(t0), dtype=np.float32),
            "c1c": c1col,
            "bc": bcol,
        })

    kw = {}
    if _trace:
        kw = dict(trace=True,
                  trace_cores=_trace_cores if _trace_cores is not None else [0])
    res = run_bass_kernel_spmd(nc, in_maps, core_ids=list(range(NCORES)), **kw)
    LAST_RESULTS = res
    LAST_EXEC_NS = res.exec_time_ns

    out = np.empty(NOUT, dtype=np.float32)
    for core in range(NCORES):
        t0 = core * KPC
        k = KPC + (HS if core == NCORES - 1 else 0)
        out[t0:t0 + k] = res.results[core]["yo"].reshape(-1)[:k]
    return out


# revision 4
# speedup vs baseline: 1.1524x; 1.1524x over previous
"""Trainium2 Bass kernel for nn_LowPassFilter (time-varying 9-tap windowed-sinc).

Math (matches reference.py):
  t in [0, N+HS):  ang = fl32(beta * t)           (f32 product rounding replicated!)
  s = sin(ang);  c = C0 + C1*s   (C0 = 4*pi^2, C1 = alpha*4000*pi)
  taps: filt[4] = 2c, filt[4+-m] = kappa_m * sin(2*pi*m*c),  kappa_m = w_{4+m}/(pi*m)
  out[t] = (2c*x[t] + sum_m filt_m*(x[t-m]+x[t+m])) / (2c + 2*sum_m filt_m)
Multiple angles from ONE pair of LUT sins (Sin LUT valid only on [-pi, pi]):
  f = c - round(c);  S1 = sin(2*pi*f) = sin(2*pi*c);  T = sin(pi*f)
  cos(2*pi*c) = 1 - 2*T^2;  sin(4*pi*c) = 2*S1*cos;  sin(6*pi*c) = S1*(3-4*S1^2)

Sharding: 1-D sequence parallel, 8 cores x 500_000 outputs (core 7: +4 tail),
halos passed from host (full input available). Layout [128 partitions x F=3968],
t_local = p*F + j, processed in 4 free-dim chunks of 992.
"""

import math
import numpy as np

# ---------------- problem constants (hardcoded per contract) ----------------
N = 4_000_000
HS = 4
NOUT = N + HS
NCORES = 8
KPC = N // NCORES            # 500_000 outputs per core (core 7 gets +HS tail)
P = 128
F = 3968                     # per-partition free size: 128*F = 507_904 >= 500_004
CH = 992                     # chunk of free dim
NCH = F // CH                # 4
CUTOFF = 1000.0
FS = 8000.0

MAGIC = 12582912.0           # 1.5 * 2**23, round-to-int magic for |v| < 2**22
C0 = float(np.float32(4.0 * math.pi * math.pi))
INV2PI = float(np.float32(1.0 / (2.0 * math.pi)))
PI_F = float(np.float32(math.pi))
TWO_PI_F = float(np.float32(2.0 * math.pi))

_W5 = math.sin(5.0 * math.pi / 8.0) ** 2     # 0.853553...
_W6 = 0.5
_W7 = math.sin(7.0 * math.pi / 8.0) ** 2     # 0.146446...
K1 = _W5 / math.pi
K2 = _W6 / (2.0 * math.pi)
K3 = _W7 / (3.0 * math.pi)
KG = float(np.float32(K1 + 2.0 * K2 + 3.0 * K3))
SQ2 = float(np.float32(2.0 * math.sqrt(K2)))  # Square(SQ2*T)   = 4*K2*T^2
SQ3 = float(np.float32(2.0 * math.sqrt(K3)))  # Square(SQ3*S1)  = 4*K3*S1^2
K1_F = float(np.float32(K1))
K2x2 = float(np.float32(2.0 * K2))
K3x3 = float(np.float32(3.0 * K3))

# Cody-Waite 3-term split of 2*pi (11-bit chunks: k <= 6366 < 2^13 keeps k*cw exact)
def _split_f32(v, bits):
    f = np.float32(v)
    m, e = math.frexp(float(f))
    scale = 2.0 ** (e - bits)
    hi = math.floor(float(f) / scale) * scale
    return float(np.float32(hi))

_TWO_PI = 2.0 * math.pi
CW1 = _split_f32(_TWO_PI, 11)
CW2 = _split_f32(_TWO_PI - CW1, 11)
CW3 = float(np.float32(_TWO_PI - CW1 - CW2))

_PROGRAM = None   # (nc,) built once per process
LAST_EXEC_NS = None
LAST_RESULTS = None


def _build_program():
    import concourse.bacc as bacc
    import concourse.mybir as mybir
    from concourse.tile import TileContext

    dt = mybir.dt.float32
    dth = mybir.dt.float16
    Alu = mybir.AluOpType
    Act = mybir.ActivationFunctionType

    nc = bacc.Bacc(None, target_bir_lowering=False, debug=False)

    xw = nc.dram_tensor("xw", [P, F + 6], dt, kind="ExternalInput")
    t0c = nc.dram_tensor("t0c", [P, 1], dt, kind="ExternalInput")
    c1c = nc.dram_tensor("c1c", [P, 1], dt, kind="ExternalInput")
    bc = nc.dram_tensor("bc", [P, 1], dt, kind="ExternalInput")
    yo = nc.dram_tensor("yo", [P, F], dt, kind="ExternalOutput")

    with TileContext(nc) as tc:
        with (
            tc.tile_pool(name="const", bufs=1) as cpool,
            tc.tile_pool(name="work", bufs=2) as pool,
        ):
            xt = cpool.tile([P, F + 6], dt, tag="xt", name="xt")
            nc.sync.dma_start(xt[:], xw[:])
            t0t = cpool.tile([P, 1], dt, tag="t0t", name="t0t")
            nc.sync.dma_start(t0t[:], t0c[:])
            c1t = cpool.tile([P, 1], dt, tag="c1t", name="c1t")
            nc.sync.dma_start(c1t[:], c1c[:])
            bt = cpool.tile([P, 1], dt, tag="bt", name="bt")
            nc.sync.dma_start(bt[:], bc[:])

            for ic in range(NCH):
                j0 = ic * CH

                TAIL = {"negG", "DhS", "Dh", "r0", "e1", "e2", "e3",
                        "nP2", "nP3", "W1", "Wt", "Z", "Y", "NUM", "o"}

                def tile(tag, d=dt):
                    return pool.tile([P, CH], d, tag=tag, name=tag,
                                     bufs=1 if tag in TAIL else 2)

                # t_local = p*F + (j0 + jj); f32 exact (< 2^24)
                tf = pool.tile([P, CH], dt, tag="tf", name="tf", bufs=2)
                nc.gpsimd.iota(tf[:], pattern=[[1, CH]], base=j0, channel_multiplier=F,
                               allow_small_or_imprecise_dtypes=True)

                # ang = fl32(beta * t)  (t = t_local + t0 exact in f32)
                ang = tile("ang")
                nc.vector.tensor_scalar(ang[:], tf[:], t0t[:, 0:1], bt[:, 0:1],
                                        Alu.add, Alu.mult)
                # k = round(ang / 2pi) via magic
                k1t = tile("k1")
                nc.scalar.activation(k1t[:], ang[:], Act.Copy, bias=MAGIC,
                                     scale=INV2PI)
                kf = tile("kf")
                nc.scalar.activation(kf[:], k1t[:], Act.Copy, bias=-MAGIC)
                # r = ((ang - k*CW1) - k*CW2) - k*CW3  in [-pi, pi]
                r = tile("r")
                nc.vector.cody_waite_cascade(r[:], ang[:], kf[:], CW1, CW2, CW3)
                s = tile("s")
                nc.scalar.activation(s[:], r[:], Act.Sin)
                # c = C0 + C1*s ; f = c - round(c)
                c = tile("c")
                nc.scalar.activation(c[:], s[:], Act.Copy, bias=C0,
                                     scale=c1t[:, 0:1])
                rk = tile("rk")
                nc.vector.tensor_scalar(rk[:], c[:], MAGIC, MAGIC,
                                        Alu.add, Alu.subtract)
                f = tile("f")
                nc.vector.tensor_tensor(f[:], c[:], rk[:], Alu.subtract)
                # trig of f
                T = tile("T", dth)
                nc.scalar.activation(T[:], f[:], Act.Sin, scale=PI_F)
                S1 = tile("S1", dth)
                nc.scalar.activation(S1[:], f[:], Act.Sin, scale=TWO_PI_F)
                Up = tile("Up", dth)      # 4*K2*sin(pi f)^2
                nc.scalar.activation(Up[:], T[:], Act.Square, scale=SQ2)
                Qp = tile("Qp", dth)      # 4*K3*sin(2pi f)^2
                nc.scalar.activation(Qp[:], S1[:], Act.Square, scale=SQ3)

                # negG = (Up - KG) + Qp = -(K1 + A2 + A3)
                negG = tile("negG", dth)
                nc.vector.scalar_tensor_tensor(negG[:], Up[:], KG, Qp[:],
                                               Alu.subtract, Alu.add)
                DhS = tile("DhS", dth)
                nc.vector.tensor_tensor(DhS[:], S1[:], negG[:], Alu.mult)
                Dh = tile("Dh")
                nc.vector.tensor_tensor(Dh[:], c[:], DhS[:], Alu.subtract)
                r0 = tile("r0")
                nc.vector.reciprocal_approx_fast(r0[:], Dh[:])

                # x side (fp16 copy of the x window incl. 6 halo cols)
                xh = pool.tile([P, CH + 6], dth, tag="xh", name="xh", bufs=1)
                nc.vector.tensor_copy(xh[:], xt[:, j0:j0 + CH + 6])
                e1 = tile("e1", dth)
                nc.vector.tensor_tensor(e1[:], xh[:, 4:4 + CH],
                                        xh[:, 2:2 + CH], Alu.add)
                e2 = tile("e2", dth)
                nc.vector.tensor_tensor(e2[:], xh[:, 5:5 + CH],
                                        xh[:, 1:1 + CH], Alu.add)
                e3 = tile("e3", dth)
                nc.vector.tensor_tensor(e3[:], xh[:, 6:6 + CH],
                                        xh[:, 0:0 + CH], Alu.add)
                nP2 = tile("nP2", dth)    # (Up - 2K2)*e2 = -A2*e2
                nc.vector.scalar_tensor_tensor(nP2[:], Up[:], K2x2, e2[:],
                                               Alu.subtract, Alu.mult)
                nP3 = tile("nP3", dth)    # (Qp - 3K3)*e3 = -A3*e3
                nc.vector.scalar_tensor_tensor(nP3[:], Qp[:], K3x3, e3[:],
                                               Alu.subtract, Alu.mult)
                W1 = tile("W1", dth)      # K1*e1 + A2*e2
                nc.vector.scalar_tensor_tensor(W1[:], e1[:], K1_F, nP2[:],
                                               Alu.mult, Alu.subtract)
                Wt = tile("Wt", dth)      # + A3*e3
                nc.vector.tensor_tensor(Wt[:], W1[:], nP3[:], Alu.subtract)
                Z = tile("Z", dth)
                nc.vector.tensor_tensor(Z[:], S1[:], Wt[:], Alu.mult)
                Y = tile("Y")
                nc.vector.tensor_tensor(Y[:], c[:], xt[:, j0 + 3:j0 + 3 + CH],
                                        Alu.mult)
                NUM = tile("NUM")    # 0.5*Z + Y
                nc.vector.scalar_tensor_tensor(NUM[:], Z[:], 0.5, Y[:],
                                               Alu.mult, Alu.add)
                o = tile("o")
                nc.vector.tensor_tensor(o[:], NUM[:], r0[:], Alu.mult)
                nc.sync.dma_start(yo[:, j0:j0 + CH], o[:])

    nc.compile()
    return nc


def _get_program():
    global _PROGRAM
    if _PROGRAM is None:
        _PROGRAM = _build_program()
    return _PROGRAM


def kernel(x, alpha, beta, _trace=False, _trace_cores=None):
    global LAST_EXEC_NS, LAST_RESULTS
    from concourse.bass_utils import run_bass_kernel_spmd

    x = np.asarray(x, dtype=np.float32).reshape(-1)
    assert x.shape[0] == N, x.shape
    a64 = float(np.float32(np.asarray(alpha).reshape(())))
    b64 = float(np.float32(np.asarray(beta).reshape(())))
    C1 = float(np.float32(a64 * 4000.0 * math.pi))

    nc = _get_program()

    # host-side shard prep: windows with halos (halo exchange via full input)
    xp = np.zeros(3 + N + (NCORES * 0 + P * F + 6), dtype=np.float32)
    xp[3:3 + N] = x
    sw = np.lib.stride_tricks.sliding_window_view(xp, F + 6)
    c1col = np.full((P, 1), C1, dtype=np.float32)
    bcol = np.full((P, 1), np.float32(b64), dtype=np.float32)
    in_maps = []
    for core in range(NCORES):
        t0 = core * KPC
        rows = np.ascontiguousarray(sw[t0 + np.arange(P) * F])   # [P, F+6]
        in_maps.append({
            "xw": rows,
            "t0c": np.full((P, 1), np.float32(t0), dtype=np.float32),
            "c1c": c1col,
            "bc": bcol,
        })

    kw = {}
    if _trace:
        kw = dict(trace=True,
                  trace_cores=_trace_cores if _trace_cores is not None else [0])
    res = run_bass_kernel_spmd(nc, in_maps, core_ids=list(range(NCORES)), **kw)
    LAST_RESULTS = res
    LAST_EXEC_NS = res.exec_time_ns

    out = np.empty(NOUT, dtype=np.float32)
    for core in range(NCORES):
        t0 = core * KPC
        k = KPC + (HS if core == NCORES - 1 else 0)
        out[t0:t0 + k] = res.results[core]["yo"].reshape(-1)[:k]
    return out
